# revision 1
# baseline (speedup 1.0000x reference)
"""Single-head causal self-attention (B=4, T=4096, C=1024, H=64) on 8 trn2 cores.

Sharding: core = (b, h) with b = core >> 1, h = core & 1. Batch b is data-parallel;
within a batch the two cores split Q rows by interleaved 512-row blocks
(core h owns global blocks {h, 2+h, 4+h, 6+h}) for causal load balance.

SPMD trick: all cores run an identical program. The host permutes x rows so each
core's own Q blocks are local rows 0..2047 (followed by the remaining blocks
ascending). Causal masking then only needs (a) fixed triangular masks for the
diagonal band (identical on every core) and (b) one per-core 0/1 scalar for the
single ambiguous 512-block per virtual block (0 for h=0, 1 for h=1).

Per-core dataflow:
  x tiles -> PE transpose -> x^T -> [Wk|Wv]-packed + Wq matmuls (fp32r)
  -> k^T [64,4096], v^T -> v_aug [s,65] (ones column for softmax denominator),
     q^T [64,2048]
  attention: S^T[s,t] = k_chunk @ q^T (PE), exp on ScalarE (scale=C^-0.5 folded),
  diag/amb masks on DVE, PV: acc[65,t] += v_aug^T @ P^T (PE) accumulating the
  softmax denominator in row 64; normalize, PE-transpose back, DMA out.

All matmuls run in float32r (full-rate fp32 streaming). Tensors consumed by
f32r matmuls are allocated as float32r so their producers round on write.
"""

import sys

if "/opt/trn_rl_repo" not in sys.path:
    sys.path.insert(0, "/opt/trn_rl_repo")

import numpy as np

import concourse.bass as bass
import concourse.mybir as mybir
from concourse import bacc
from concourse.tile import TileContext
from concourse.masks import make_identity

B, T, C, H = 4, 4096, 1024, 64
NCORES = 8
TB = 512            # virtual t-block size
NB = T // (2 * TB)  # 4 virtual blocks per core
SC = 128            # s-chunk size
NCC = C // 128      # 8 contraction chunks
F32 = mybir.dt.float32
F32R = mybir.dt.float32r
SCALE = float(C) ** -0.5

_CACHED_NC = {}


def build_module(repeat=1):
    nc = bacc.Bacc("TRN2", target_bir_lowering=False)
    x_d = nc.dram_tensor("x", [T, C], F32, kind="ExternalInput")
    wk_d = nc.dram_tensor("wk", [C, H], F32, kind="ExternalInput")
    wq_d = nc.dram_tensor("wq", [C, H], F32, kind="ExternalInput")
    wv_d = nc.dram_tensor("wv", [C, H], F32, kind="ExternalInput")
    amb_d = nc.dram_tensor("amb", [128, 1], F32, kind="ExternalInput")
    out_d = nc.dram_tensor("out", [T // 2, H], F32, kind="ExternalOutput")

    with TileContext(nc) as tc:
        with (
            tc.tile_pool(name="const", bufs=1) as const,
            tc.tile_pool(name="xin", bufs=4) as xin,
            tc.tile_pool(name="xt", bufs=2) as xtp,
            tc.tile_pool(name="proj", bufs=1) as proj,
            tc.tile_pool(name="vstage", bufs=2) as vstage,
            tc.tile_pool(name="pt", bufs=3) as ptp,
            tc.tile_pool(name="outp", bufs=2) as outp,
            tc.tile_pool(name="ps_tr", bufs=2, space="PSUM") as ps_tr,
            tc.tile_pool(name="ps_kvq", bufs=1, space="PSUM") as ps_kvq,
            tc.tile_pool(name="ps_s", bufs=2, space="PSUM") as ps_s,
            tc.tile_pool(name="ps_acc", bufs=1, space="PSUM") as ps_acc,
        ):
            # ---------------- constants ----------------
            ident = const.tile([128, 128], F32)
            make_identity(nc, ident)

            # tri[j][s, t] = 1.0 iff t >= s + 128j  (t: free 0..511, s: partition)
            tristage = const.tile([128, 4, TB], F32)
            nc.gpsimd.memset(tristage, 1.0)
            for j in range(4):
                nc.gpsimd.affine_select(
                    out=tristage[:, j, :],
                    in_=tristage[:, j, :],
                    compare_op=mybir.AluOpType.is_ge,
                    fill=0.0,
                    base=-128 * j,
                    pattern=[[1, TB]],
                    channel_multiplier=-1,
                )
            tri = const.tile([128, 4, TB], F32R)
            nc.vector.tensor_copy(out=tri, in_=tristage)

            amb = const.tile([128, 1], F32)
            nc.sync.dma_start(out=amb, in_=amb_d[:, :])

            # packed stationary weights: wkv[:, ci, 0:64] = Wk chunk, [...,64:128] = Wv
            wstage = const.tile([128, NCC, 128], F32)
            wqstage = const.tile([128, NCC, H], F32)
            for ci in range(NCC):
                nc.sync.dma_start(out=wstage[:, ci, 0:H], in_=wk_d[128 * ci:128 * (ci + 1), :])
                nc.sync.dma_start(out=wstage[:, ci, H:128], in_=wv_d[128 * ci:128 * (ci + 1), :])
                nc.sync.dma_start(out=wqstage[:, ci, :], in_=wq_d[128 * ci:128 * (ci + 1), :])
            wkv = const.tile([128, NCC, 128], F32R)
            wq = const.tile([128, NCC, H], F32R)
            nc.vector.tensor_copy(out=wkv, in_=wstage)
            nc.vector.tensor_copy(out=wq, in_=wqstage)

            # ---------------- persistent activations ----------------
            for _rep in range(repeat):
              kT = proj.tile([64, T], F32R)       # k^T, perm order
              qT = proj.tile([64, T // 2], F32R)  # q^T, own rows
              vaug = proj.tile([128, T // SC, H + 1], F32R)  # v_aug[s_chunk][s, 65]
              onecol = const.tile([128, 1], F32)
              nc.gpsimd.memset(onecol, 1.0)
              nc.vector.tensor_copy(
                  out=vaug[:, :, H:H + 1], in_=onecol.to_broadcast((128, T // SC, 1))
              )

              # ---------------- phase B + C interleaved ----------------
              # B group order [0,4,1,5,...]: C block k needs pos-blocks 0..k and
              # 4..4+k, so emit those groups first and interleave C emission.
              def emit_b_group(g):
                  xt = xtp.tile([128, NCC, TB], F32R)  # x^T group: [c, ci, t]
                  for j in range(TB // 128):  # 4 x-tiles of 128 rows
                      xtile = xin.tile([128, C], F32)
                      row0 = TB * g + 128 * j
                      nc.sync.dma_start(out=xtile, in_=x_d[row0:row0 + 128, :])
                      for half in range(2):  # 4 c-chunks per psum batch
                          trp = ps_tr.tile([128, 4, 128], F32, tag="tr")
                          for m in range(4):
                              ci = 4 * half + m
                              nc.tensor.transpose(
                                  trp[:, m, :], xtile[:, 128 * ci:128 * (ci + 1)], ident
                              )
                          nc.vector.tensor_copy(
                              xt[:, 4 * half:4 * half + 4, 128 * j:128 * (j + 1)], trp
                          )

                  kv = ps_kvq.tile([128, TB], F32, tag="kvq")
                  for ci in range(NCC):
                      nc.tensor.matmul(
                          kv, wkv[:, ci, :], xt[:, ci, :],
                          start=(ci == 0), stop=(ci == NCC - 1),
                      )
                  nc.scalar.copy(out=kT[:, TB * g:TB * (g + 1)], in_=kv[0:64, :])
                  vT = vstage.tile([64, TB], F32)
                  nc.vector.tensor_copy(out=vT, in_=kv[64:128, :])
                  # v^T -> v natural [s, h] chunks
                  vtp = ps_tr.tile([128, 4, H], F32, tag="tr")
                  for m in range(4):
                      nc.tensor.transpose(
                          vtp[:, m, :], vT[:, 128 * m:128 * (m + 1)], ident[0:64, 0:64]
                      )
                  nc.vector.tensor_copy(out=vaug[:, 4 * g:4 * g + 4, 0:H], in_=vtp)

                  if g < NB:  # q projection: own rows only (local groups 0..3)
                      qp = ps_kvq.tile([64, TB], F32, tag="kvq")
                      for ci in range(NCC):
                          nc.tensor.matmul(
                              qp, wq[:, ci, :], xt[:, ci, :],
                              start=(ci == 0), stop=(ci == NCC - 1),
                          )
                      nc.scalar.copy(out=qT[:, TB * g:TB * (g + 1)], in_=qp)

              def emit_c_block(k):
                  qs = qT[:, TB * k:TB * (k + 1)]
                  # s-chunks (128 rows each) in perm position space:
                  own = list(range(0, 4 * (k + 1)))          # blocks 0..k
                  rest = list(range(16, 16 + 4 * (k + 1)))   # rest blocks 0..k
                  chunks = own + rest
                  acc = ps_acc.tile([H + 1, TB], F32)
                  n = len(chunks)
                  for p0 in range(0, n, 2):  # pairs of chunks share a psum tile
                      pair = chunks[p0:p0 + 2]
                      st = ps_s.tile([128, 2 * TB], F32)
                      for i, ch in enumerate(pair):
                          nc.tensor.matmul(
                              st[:, TB * i:TB * (i + 1)],
                              kT[:, SC * ch:SC * (ch + 1)], qs,
                              start=True, stop=True,
                          )
                      pt = ptp.tile([128, 2 * TB], F32R)
                      nc.scalar.activation(
                          out=pt, in_=st, func=mybir.ActivationFunctionType.Exp,
                          scale=SCALE,
                      )
                      for i, ch in enumerate(pair):
                          pslice = pt[:, TB * i:TB * (i + 1)]
                          if ch in own[-4:]:  # diagonal band: triangular mask
                              j = ch - 4 * k
                              nc.vector.tensor_mul(pslice, pslice, tri[:, j, :])
                          elif ch in rest[-4:]:  # ambiguous block: 0/1 per core
                              nc.vector.tensor_scalar_mul(pslice, pslice, amb[:, 0:1])
                      for i, ch in enumerate(pair):
                          nc.tensor.matmul(
                              acc, vaug[:, ch, :], pt[:, TB * i:TB * (i + 1)],
                              start=(p0 == 0 and i == 0), stop=(p0 + i == n - 1),
                          )

                  # normalize + transpose back + store
                  accs = outp.tile([H + 1, TB], F32, tag="accs")
                  nc.vector.tensor_copy(out=accs, in_=acc)
                  otp = ps_tr.tile([128, 4, H + 1], F32, tag="tr")
                  for m in range(4):
                      nc.tensor.transpose(
                          otp[:, m, :], accs[:, 128 * m:128 * (m + 1)],
                          ident[0:H + 1, 0:H + 1],
                      )
                  ob = outp.tile([128, 4, H + 1], F32, tag="ob")
                  nc.vector.tensor_copy(out=ob, in_=otp)
                  of = outp.tile([128, 4, H], F32, tag="of")
                  rec = outp.tile([128, 4], F32, tag="rec")
                  for m in range(4):
                      nc.vector.reciprocal(rec[:, m:m + 1], ob[:, m, H:H + 1])
                      nc.vector.tensor_scalar_mul(of[:, m, :], ob[:, m, 0:H], rec[:, m:m + 1])
                  nc.sync.dma_start(
                      out=out_d[TB * k:TB * (k + 1), :].rearrange("(m p) h -> p m h", p=128),
                      in_=of,
                  )

              for idx, g in enumerate([0, 4, 1, 5, 2, 6, 3, 7]):
                  emit_b_group(g)
                  if idx % 2 == 1:
                      emit_c_block(idx // 2)

    nc.compile()
    return nc


def _get_nc(repeat=1):
    if repeat not in _CACHED_NC:
        _CACHED_NC[repeat] = build_module(repeat)
    return _CACHED_NC[repeat]


def _perm_blocks(h):
    own = [h + 2 * i for i in range(4)]
    rest = [(1 - h) + 2 * i for i in range(4)]
    return own, rest


def make_in_maps(x, wk, wq, wv):
    in_maps = []
    for core in range(NCORES):
        b, h = core >> 1, core & 1
        own, rest = _perm_blocks(h)
        rows = np.concatenate(
            [np.arange(TB * g, TB * (g + 1)) for g in own + rest]
        )
        in_maps.append({
            "x": np.ascontiguousarray(x[b][rows]),
            "wk": wk, "wq": wq, "wv": wv,
            "amb": np.full((128, 1), float(h), dtype=np.float32),
        })
    return in_maps


def assemble(results):
    out = np.empty((B, T, H), dtype=np.float32)
    for core in range(NCORES):
        b, h = core >> 1, core & 1
        own, _ = _perm_blocks(h)
        o = results[core]["out"]
        for k, g in enumerate(own):
            out[b, TB * g:TB * (g + 1), :] = o[TB * k:TB * (k + 1), :]
    return out


def kernel(x, Wk, Wq, Wv):
    from concourse import bass_utils

    x = np.asarray(x, dtype=np.float32)
    wk = np.ascontiguousarray(np.asarray(Wk, dtype=np.float32))
    wq = np.ascontiguousarray(np.asarray(Wq, dtype=np.float32))
    wv = np.ascontiguousarray(np.asarray(Wv, dtype=np.float32))
    nc = _get_nc()
    in_maps = make_in_maps(x, wk, wq, wv)
    res = bass_utils.run_bass_kernel_spmd(nc, in_maps, core_ids=list(range(NCORES)))
    return assemble(res.results)



# revision 21
# speedup vs baseline: 8.3445x; 8.3445x over previous
"""Single-head causal self-attention (B=4, T=4096, C=1024, H=64) on 8 trn2 cores.

Sharding: core = (b, h) with b = core >> 1, h = core & 1. Batch b is data-parallel;
within a batch the two cores split Q rows by interleaved 512-row blocks
(core h owns global blocks {h, 2+h, 4+h, 6+h}) for causal load balance.

SPMD trick: all cores run an identical program. The host ships x already
TRANSPOSED (x^T, [C, T]) in bf16 with rows permuted so each core's own Q blocks
are local t 0..2047 (remaining blocks follow, ascending). Causal masking needs
only (a) fixed triangular selects for the diagonal band (identical on every
core, applied in-place by GpSimd affine_select) and (b) one per-core exp-bias
scalar (0 or -100) folded into the softmax exp for the single ambiguous
512-block per virtual block.

Per-core dataflow (all matmul operands bf16, fp32 PSUM accumulation):
  x^T tiles stream from DRAM -> [Wk|Wv]-packed and [Wq|Wq]-packed matmuls
  -> kv psum (rows 0:64 k^T, 64:128 v^T); k^T copied twice into kTdup
  [128, T] (row-half 2 shifted by 128 cols) so S-chunk pairs can run as two
  concurrent row-tiled (K=64) matmuls; [Wq|Wq] gives the duplicated q^T rows
  for free. v^T is PE-transposed into vaug [s, 65] (ones column accumulates
  the softmax denominator during PV).
  attention per 512-t block: S^T pair (PE, tile_position rows 0/64) -> exp on
  ScalarE straight out of PSUM (scale=C^-0.5, amb bias folded in) -> bf16 P in
  SBUF -> diagonal tri-mask via GpSimd affine_select in place -> PV (PE)
  accumulating [65, 512]; normalize, PE-transpose back (f32r), DMA out.
"""

import sys

if "/opt/trn_rl_repo" not in sys.path:
    sys.path.insert(0, "/opt/trn_rl_repo")

import numpy as np

import concourse.bass as bass
import concourse.mybir as mybir
from concourse import bacc
from concourse.tile import TileContext
from concourse.masks import make_identity

B, T, C, H = 4, 4096, 1024, 64
NCORES = 8
TB = 512            # t-block size
NB = T // (2 * TB)  # 4 own blocks per core
SC = 128            # s-chunk size
NCC = C // 128      # 8 contraction chunks
F32 = mybir.dt.float32
F32R = mybir.dt.float32r
BF16 = mybir.dt.bfloat16
SCALE = float(C) ** -0.5
AMB_OFF = -100.0    # exp(-100) == 0: masks the ambiguous block on h=0 cores

_CACHED_NC = {}


def build_module(repeat=1):
    nc = bacc.Bacc("TRN2", target_bir_lowering=False)
    xt_d = nc.dram_tensor("xt", [C, T], BF16, kind="ExternalInput")
    wkv_d = nc.dram_tensor("wkv", [128, NCC, 128], BF16, kind="ExternalInput")
    wq2_d = nc.dram_tensor("wq2", [128, NCC, 128], BF16, kind="ExternalInput")
    ambb_d = nc.dram_tensor("ambb", [128, 1], F32, kind="ExternalInput")
    out_d = nc.dram_tensor("out", [T // 2, H], F32, kind="ExternalOutput")

    with TileContext(nc) as tc:
        with (
            tc.tile_pool(name="const", bufs=1) as const,
            tc.tile_pool(name="xin", bufs=6) as xin,
            tc.tile_pool(name="proj", bufs=1) as proj,
            tc.tile_pool(name="kvst", bufs=2) as kvst,
            tc.tile_pool(name="ptp", bufs=3) as ptp,
            tc.tile_pool(name="outp", bufs=2) as outp,
            tc.tile_pool(name="ps_s", bufs=2, space="PSUM") as ps_s,
            tc.tile_pool(name="ps_kvq", bufs=2, space="PSUM") as ps_kvq,
            tc.tile_pool(name="ps_acc", bufs=1, space="PSUM") as ps_acc,
            tc.tile_pool(name="ps_tr", bufs=1, space="PSUM") as ps_tr,
        ):
            # ---------------- constants ----------------
            ident_b = const.tile([128, 128], BF16)
            make_identity(nc, ident_b)
            ident_s = const.tile([128, 128], F32)
            make_identity(nc, ident_s)

            wkv = const.tile([128, NCC, 128], BF16)
            wq2 = const.tile([128, NCC, 128], BF16)
            ambb = const.tile([128, 1], F32)
            nc.sync.dma_start(out=wkv, in_=wkv_d[:, :, :])
            nc.sync.dma_start(out=wq2, in_=wq2_d[:, :, :])
            nc.sync.dma_start(out=ambb, in_=ambb_d[:, :])

            for _rep in range(repeat):
                # ---------------- persistent activations ----------------
                # kTdup rows 0:64 = k^T; rows 64:128 = k^T shifted left 128 cols
                # (pair i stationaries: [0:64, 256i:256i+128] = chunk 2i,
                #  [64:128, 256i:256i+128] = chunk 2i+1).
                kTdup = proj.tile([128, T], BF16)
                qdup = proj.tile([128, T // 2], BF16)  # rows 0:64 == 64:128 == q^T
                vaug = proj.tile([128, T // SC, H + 1], BF16)
                nc.gpsimd.memset(vaug[:, :, H:H + 1], 1.0)

                def emit_group(g):
                    # x^T tile for perm block g: [c-part, ci, t]
                    xt = xin.tile([128, NCC, TB], BF16)
                    for ci in range(NCC):
                        nc.sync.dma_start(
                            out=xt[:, ci, :],
                            in_=xt_d[128 * ci:128 * (ci + 1), TB * g:TB * (g + 1)],
                        )
                    kvp = ps_kvq.tile([128, TB], F32, tag="kvq")
                    for ci in range(NCC):
                        nc.tensor.matmul(
                            kvp, wkv[:, ci, :], xt[:, ci, :],
                            start=(ci == 0), stop=(ci == NCC - 1),
                        )
                    nc.vector.tensor_copy(
                        out=kTdup[0:64, TB * g:TB * (g + 1)], in_=kvp[0:64, :]
                    )
                    if g == 0:
                        nc.vector.tensor_copy(
                            out=kTdup[64:128, 0:TB - SC], in_=kvp[0:64, SC:TB]
                        )
                    else:
                        nc.vector.tensor_copy(
                            out=kTdup[64:128, TB * g - SC:TB * g + (TB - SC)],
                            in_=kvp[0:64, :],
                        )
                    vts = kvst.tile([64, TB], BF16)
                    nc.vector.tensor_copy(out=vts, in_=kvp[64:128, :])
                    # v^T -> v natural [s, h] chunks
                    vtp = ps_tr.tile([128, 4, H], BF16, tag="tr")
                    for m in range(4):
                        nc.tensor.transpose(
                            vtp[:, m, :],
                            vts[:, SC * m:SC * (m + 1)],
                            ident_b[0:64, 0:64],
                        )
                    nc.vector.tensor_copy(
                        out=vaug[:, 4 * g:4 * g + 4, 0:H], in_=vtp
                    )

                    if g < NB:  # q projection: own rows only
                        qp = ps_kvq.tile([128, TB], F32, tag="kvq")
                        for ci in range(NCC):
                            nc.tensor.matmul(
                                qp, wq2[:, ci, :], xt[:, ci, :],
                                start=(ci == 0), stop=(ci == NCC - 1),
                            )
                        nc.vector.tensor_copy(
                            out=qdup[:, TB * g:TB * (g + 1)], in_=qp
                        )

                def emit_block(k):
                    qs1 = qdup[0:64, TB * k:TB * (k + 1)]
                    qs2 = qdup[64:128, TB * k:TB * (k + 1)]
                    # pair i covers s-chunks (2i, 2i+1); own pairs then rest
                    pairs = list(range(2 * (k + 1))) + [
                        8 + i for i in range(2 * (k + 1))
                    ]
                    n = len(pairs)
                    acc = ps_acc.tile([H + 1, TB], F32)
                    for idx, i in enumerate(pairs):
                        st = ps_s.tile([128, 2, TB], F32)
                        nc.tensor.matmul(
                            st[:, 0, :], kTdup[0:64, 256 * i:256 * i + 128],
                            qs1, start=True, stop=True,
                        )
                        nc.tensor.matmul(
                            st[:, 1, :], kTdup[64:128, 256 * i:256 * i + 128],
                            qs2, start=True, stop=True,
                        )
                        pt = ptp.tile([128, 2, TB], BF16)
                        amb = i >= 8 and (i - 8) >= 2 * k
                        nc.scalar.activation(
                            out=pt, in_=st, func=mybir.ActivationFunctionType.Exp,
                            scale=SCALE, bias=ambb[:, 0:1] if amb else 0.0,
                        )
                        if i < 8 and i >= 2 * k:  # diagonal band: tri select
                            for c in range(2):
                                j = 2 * i + c - 4 * k
                                nc.gpsimd.affine_select(
                                    out=pt[:, c, :], in_=pt[:, c, :],
                                    compare_op=mybir.AluOpType.is_ge,
                                    fill=0.0, base=-SC * j,
                                    pattern=[[1, TB]], channel_multiplier=-1,
                                )
                        for c in range(2):
                            nc.tensor.matmul(
                                acc, vaug[:, 2 * i + c, :], pt[:, c, :],
                                start=(idx == 0 and c == 0),
                                stop=(idx == n - 1 and c == 1),
                            )

                    # normalize + transpose back + store
                    accs = outp.tile([H + 1, TB], F32, tag="accs")
                    nc.vector.tensor_copy(out=accs, in_=acc)
                    otp = ps_tr.tile([128, 4, H + 1], F32, tag="tr")
                    for m in range(4):
                        nc.tensor.transpose(
                            otp[:, m, :], accs[:, SC * m:SC * (m + 1)],
                            ident_s[0:H + 1, 0:H + 1],
                        )
                    ob = outp.tile([128, 4, H + 1], F32, tag="ob")
                    nc.vector.tensor_copy(out=ob, in_=otp)
                    rec = outp.tile([128, 4], F32, tag="rec")
                    nc.vector.reciprocal(rec, ob[:, :, H])
                    of = outp.tile([128, 4, H], F32, tag="of")
                    for m in range(4):
                        nc.vector.tensor_scalar_mul(
                            of[:, m, :], ob[:, m, 0:H], rec[:, m:m + 1]
                        )
                    nc.sync.dma_start(
                        out=out_d[TB * k:TB * (k + 1), :].rearrange(
                            "(m p) h -> p m h", p=128
                        ),
                        in_=of,
                    )

                for idx, g in enumerate([0, 4, 1, 5, 2, 6, 3, 7]):
                    emit_group(g)
                    if idx % 2 == 1:
                        emit_block(idx // 2)

    nc.compile()
    return nc


def _get_nc(repeat=1):
    if repeat not in _CACHED_NC:
        _CACHED_NC[repeat] = build_module(repeat)
    return _CACHED_NC[repeat]


def _perm_blocks(h):
    own = [h + 2 * i for i in range(4)]
    rest = [(1 - h) + 2 * i for i in range(4)]
    return own, rest


def make_in_maps(x, wk, wq, wv):
    import ml_dtypes

    bf16 = ml_dtypes.bfloat16
    wkv = np.concatenate([wk, wv], axis=1).astype(bf16)   # [C, 128]
    wkv = np.ascontiguousarray(wkv.reshape(NCC, 128, 128).transpose(1, 0, 2))
    wq2 = np.concatenate([wq, wq], axis=1).astype(bf16)
    wq2 = np.ascontiguousarray(wq2.reshape(NCC, 128, 128).transpose(1, 0, 2))
    in_maps = []
    for core in range(NCORES):
        b, h = core >> 1, core & 1
        own, rest = _perm_blocks(h)
        rows = np.concatenate(
            [np.arange(TB * g, TB * (g + 1)) for g in own + rest]
        )
        in_maps.append({
            "xt": np.ascontiguousarray(x[b][rows].T.astype(bf16)),
            "wkv": wkv, "wq2": wq2,
            "ambb": np.full(
                (128, 1), 0.0 if h == 1 else AMB_OFF, dtype=np.float32
            ),
        })
    return in_maps


def assemble(results):
    out = np.empty((B, T, H), dtype=np.float32)
    for core in range(NCORES):
        b, h = core >> 1, core & 1
        own, _ = _perm_blocks(h)
        o = results[core]["out"]
        for k, g in enumerate(own):
            out[b, TB * g:TB * (g + 1), :] = o[TB * k:TB * (k + 1), :]
    return out


def kernel(x, Wk, Wq, Wv):
    from concourse import bass_utils

    x = np.asarray(x, dtype=np.float32)
    wk = np.ascontiguousarray(np.asarray(Wk, dtype=np.float32))
    wq = np.ascontiguousarray(np.asarray(Wq, dtype=np.float32))
    wv = np.ascontiguousarray(np.asarray(Wv, dtype=np.float32))
    nc = _get_nc()
    in_maps = make_in_maps(x, wk, wq, wv)
    res = bass_utils.run_bass_kernel_spmd(nc, in_maps, core_ids=list(range(NCORES)))
    return assemble(res.results)
